# revision 1
# baseline (speedup 1.0000x reference)
"""TRN2 Bass kernel for nn_Construct_76484777607483.

Computes, for 12 input tensors x_i [B=2, C=256, H=64, W=256]:
    y_i = einsum('bchw,co->bohw', x_i, W)
interleaved over H (output row 12*h + i comes from tensor i, row h) into
out [2, 256, 768, 256], plus bias b[o] * count(row) where count is the
conv-transpose overlap multiplicity (ramp 1..12 at the top edge, 12 in the
middle, 12..1 at the bottom edge).

Sharding: 8 cores = (2 batches) x (4 h-quarters of 16 input rows). Each core
handles all 12 tensors for its 16 rows, so the row-interleave is assembled
on-chip and output DMA writes are fully contiguous per channel.

Per-core kernel: for each group of 2 input rows (512 pixels), for each tensor
i, a [256 -> 256] channel matmul is done as 2 accumulating 128x128x512
matmuls in float32r (full-rate PE path, ~1.5e-4 rel err), then the PSUM tile
is copied into an interleave-layout SBUF buffer with the per-(i, h) bias
value added as a per-partition scalar (DVE tensor_scalar_add). The bias
values (b[o] * count) are precomputed on host per core.
"""

import numpy as np

import concourse.bacc as bacc
import concourse.tile as tile
import concourse.mybir as mybir
from concourse.bass_utils import run_bass_kernel_spmd

B, C, H, WD = 2, 256, 64, 256
NT = 12                 # stacked tensors
NCORES = 8
HQ = H // 4             # 16 input rows per core
NG = HQ // 2            # 8 groups of 2 rows
HOUT = NT * H           # 768

_F32 = mybir.dt.float32
_F32R = mybir.dt.float32r

_NC_CACHE = {}


def build_nc():
    if "nc" in _NC_CACHE:
        return _NC_CACHE["nc"]
    nc = bacc.Bacc("TRN2", target_bir_lowering=False)
    x_d = nc.declare_dram_parameter("x", [NT, C, HQ, WD], _F32R, isOutput=False)
    w_d = nc.declare_dram_parameter("w", [C, C], _F32R, isOutput=False)
    bv_d = nc.declare_dram_parameter("bv", [2, 128, NT * HQ], _F32, isOutput=False)
    y_d = nc.declare_dram_parameter("y", [C, NT * HQ, WD], _F32, isOutput=True)

    with tile.TileContext(nc) as tc:
        with (
            tc.tile_pool(name="const", bufs=1) as cpool,
            tc.tile_pool(name="xin", bufs=6) as inpool,
            tc.tile_pool(name="obuf", bufs=3) as outpool,
            tc.tile_pool(name="ps", bufs=4, space="PSUM") as pspool,
        ):
            wt = [
                [
                    cpool.tile([128, 128], _F32R, name=f"w{kh}{mh}")
                    for mh in range(2)
                ]
                for kh in range(2)
            ]
            for kh in range(2):
                for mh in range(2):
                    # consts load on the ACT ring so the SP ring starts the
                    # first input tiles immediately
                    nc.scalar.dma_start(
                        out=wt[kh][mh][:],
                        in_=w_d[kh * 128 : (kh + 1) * 128, mh * 128 : (mh + 1) * 128],
                    )
            bvt = [cpool.tile([128, NT * HQ], _F32, name=f"bv{mh}") for mh in range(2)]
            for mh in range(2):
                nc.scalar.dma_start(out=bvt[mh][:], in_=bv_d[mh])

            for g in range(NG):
                obufs = [
                    outpool.tile(
                        [128, 2, NT, WD], _F32, name=f"ob{g}_{mh}", tag=f"ob{mh}"
                    )
                    for mh in range(2)
                ]
                for i0 in range(0, NT, 2):
                    xps = []
                    for i in (i0, i0 + 1):
                        xin = inpool.tile(
                            [128, 2, 2, WD], _F32R, name=f"xin{g}_{i}", tag="xin"
                        )
                        for kh in range(2):
                            eng = nc.gpsimd if (kh == 1 and i >= 5) else nc.sync
                            eng.dma_start(
                                out=xin[:, kh],
                                in_=x_d[
                                    i, kh * 128 : (kh + 1) * 128, 2 * g : 2 * g + 2, :
                                ],
                            )
                        xps.append(xin)
                    for mh in range(2):
                        # one 2-bank PSUM tile per tensor PAIR [128, ip, hl, WD]
                        ps = pspool.tile(
                            [128, 2, 2, WD], _F32, name=f"ps{g}_{i0}_{mh}", tag="ps"
                        )
                        for ip in range(2):
                            nc.tensor.matmul(
                                ps[:, ip],
                                wt[0][mh][:],
                                xps[ip][:, 0],
                                start=True,
                                stop=False,
                            )
                            nc.tensor.matmul(
                                ps[:, ip],
                                wt[1][mh][:],
                                xps[ip][:, 1],
                                start=False,
                                stop=True,
                            )
                        # ~1/6 of the PSUM->SBUF bias-add copies run on the
                        # ACT engine (activation Identity with per-partition
                        # bias), the rest on DVE, balancing both engines
                        on_act = i0 == 10
                        if g in (0, NG - 1):
                            # one of the two rows is the 0/63 boundary row,
                            # whose bias count varies per tensor: copy that
                            # row per tensor, merge the uniform row per pair
                            hv = 0 if g == 0 else 1  # varying-count row
                            hu = 1 - hv
                            for ip in range(2):
                                col = (i0 + ip) * HQ + 2 * g + hv
                                if on_act:
                                    nc.scalar.activation(
                                        obufs[mh][:, hv, i0 + ip],
                                        ps[:, ip, hv],
                                        mybir.ActivationFunctionType.Identity,
                                        bias=bvt[mh][:, col : col + 1],
                                    )
                                else:
                                    nc.vector.tensor_scalar_add(
                                        obufs[mh][:, hv, i0 + ip],
                                        ps[:, ip, hv],
                                        bvt[mh][:, col : col + 1],
                                    )
                            col = i0 * HQ + 2 * g + hu
                            if on_act:
                                nc.scalar.activation(
                                    obufs[mh][:, hu, i0 : i0 + 2],
                                    ps[:, :, hu],
                                    mybir.ActivationFunctionType.Identity,
                                    bias=bvt[mh][:, col : col + 1],
                                )
                            else:
                                nc.vector.tensor_scalar_add(
                                    obufs[mh][:, hu, i0 : i0 + 2],
                                    ps[:, :, hu],
                                    bvt[mh][:, col : col + 1],
                                )
                        else:
                            # interior rows: count uniform (12) across both
                            # tensors and rows -> one op per pair
                            col = i0 * HQ + 2 * g
                            src = ps[:].transpose([0, 2, 1, 3])  # (hl, ip, w)
                            if on_act:
                                nc.scalar.activation(
                                    obufs[mh][:, :, i0 : i0 + 2],
                                    src,
                                    mybir.ActivationFunctionType.Identity,
                                    bias=bvt[mh][:, col : col + 1],
                                )
                            else:
                                nc.vector.tensor_scalar_add(
                                    obufs[mh][:, :, i0 : i0 + 2],
                                    src,
                                    bvt[mh][:, col : col + 1],
                                )
                for mh in range(2):
                    # outputs split across the ACT HWDGE ring and the SWDGE
                    # (gpsimd) ring; small pieces keep each DMA-lane hold
                    # short to avoid head-of-line blocking
                    for q in range(6):
                        eng = (
                            nc.gpsimd
                            if (mh * 6 + q) in (1, 3, 5, 8, 10)
                            else nc.scalar
                        )
                        eng.dma_start(
                            out=y_d[
                                mh * 128 : (mh + 1) * 128,
                                24 * g + 4 * q : 24 * g + 4 * (q + 1),
                                :,
                            ],
                            in_=obufs[mh][:, q // 3, (q % 3) * 4 : (q % 3) * 4 + 4],
                        )
    nc.finalize()
    _NC_CACHE["nc"] = nc
    return nc


def _counts() -> np.ndarray:
    """count[r] for output row r (conv-transpose bias multiplicity)."""
    r = np.arange(HOUT)
    return (np.minimum(11, r) - np.maximum(0, r - (HOUT - NT)) + 1).astype(np.float32)


def shard_inputs(inputs: dict) -> list[dict]:
    xs = [np.ascontiguousarray(np.asarray(inputs[f"x{i}"], dtype=np.float32)) for i in range(NT)]
    w = np.ascontiguousarray(np.asarray(inputs["W"], dtype=np.float32))
    b = np.asarray(inputs["b"], dtype=np.float32)
    counts = _counts()
    in_maps = []
    for cid in range(NCORES):
        b_idx, hq = divmod(cid, 4)
        h0 = hq * HQ
        x_core = np.empty((NT, C, HQ, WD), dtype=np.float32)
        for i in range(NT):
            x_core[i] = xs[i][b_idx, :, h0 : h0 + HQ, :]
        # bv[mh, o, i*HQ + hl] = b[mh*128+o] * count(12*(h0+hl) + i)
        i_idx = np.arange(NT)[:, None]
        hl_idx = np.arange(HQ)[None, :]
        cnt = counts[12 * (h0 + hl_idx) + i_idx].reshape(NT * HQ)  # [192]
        bv = (b.reshape(2, 128)[:, :, None] * cnt[None, None, :]).astype(np.float32)
        in_maps.append({"x": x_core, "w": w, "bv": bv})
    return in_maps


def gather_outputs(results: list[dict]) -> np.ndarray:
    out = np.empty((B, C, HOUT, WD), dtype=np.float32)
    for cid in range(NCORES):
        b_idx, hq = divmod(cid, 4)
        h0 = hq * HQ
        out[b_idx, :, 12 * h0 : 12 * h0 + NT * HQ, :] = results[cid]["y"]
    return out


def kernel(**inputs) -> np.ndarray:
    nc = build_nc()
    in_maps = shard_inputs(inputs)
    res = run_bass_kernel_spmd(nc, in_maps, core_ids=list(range(NCORES)))
    return gather_outputs(res.results)



# revision 39
# speedup vs baseline: 1.3720x; 1.3720x over previous
"""TRN2 Bass kernel for nn_Construct_76484777607483.

Computes, for 12 input tensors x_i [B=2, C=256, H=64, W=256]:
    y_i = einsum('bchw,co->bohw', x_i, W)
interleaved over H (output row 12*h + i comes from tensor i, row h) into
out [2, 256, 768, 256], plus bias b[o] * count(row) where count is the
conv-transpose overlap multiplicity (ramp 1..12 at the top edge, 12 in the
middle, 12..1 at the bottom edge).

Sharding: 8 cores = (2 batches) x (4 h-quarters of 16 input rows).

Datapath is bf16 end-to-end (inputs cast on host, outputs stored bf16 and
upcast on host): bf16 matmul runs at the same 1 cycle/row PE rate as fp32r
but halves every DMA. Queue roles: SP ring carries all input DMA (75.8us),
gpsimd/SWDGE ring all output DMA (75.8us), ACT evacuates mh=0 PSUM tiles
(bias-add via activation Identity), DVE evacuates mh=1 (tensor_scalar_add);
PE at 81.9us is the bottleneck, as it should be for this compute-regime
problem.

Per (stripe s of 4 input rows, tensor i): one input DMA [128, 2kh, 1024],
two PSUM tiles (mh halves) of [128, 4, 256] each built by 2 accumulating
bf16 matmuls, evacuated with the per-(i,row) bias b[o]*count added as a
per-partition scalar, then one output DMA per mh into y[mh, :, hl, i, :]
whose (hl, i) index order IS the row interleave, so the host just reshapes.
Edge stripes (s=0 row 0, s=3 row 3) split the evac in two because the bias
count varies on the outermost output rows; the split is structural on all
cores (SPMD), only the bias table data differs.
"""

import numpy as np

import concourse.bacc as bacc
import concourse.tile as tile
import concourse.mybir as mybir
from concourse.bass_utils import run_bass_kernel_spmd

B, C, H, WD = 2, 256, 64, 256
NT = 12                 # stacked tensors
NCORES = 8
HQ = H // 4             # 16 input rows per core
NS = 4                  # stripes per core
SR = HQ // NS           # 4 input rows per stripe
HOUT = NT * H           # 768

_F32 = mybir.dt.float32
_BF16 = mybir.dt.bfloat16
_NPBF16 = mybir.dt.np(_BF16)

_NC_CACHE = {}


def build_nc(n_warm=16, first_split=True, xin_bufs=6, ob_bufs=8, ps_bufs=4, last_split=True,
             sp_tail_n=1, slb_ring="sp"):
    key = (n_warm, first_split, xin_bufs, ob_bufs, ps_bufs, last_split, sp_tail_n, slb_ring)
    if key in _NC_CACHE:
        return _NC_CACHE[key]
    nc = bacc.Bacc("TRN2", target_bir_lowering=False)
    # x[p, s, i, kh, r*WD]: channel = kh*128 + p, input row = s*SR + r
    x_d = nc.declare_dram_parameter("x", [128, NS, NT, 2, SR * WD], _BF16, isOutput=False)
    w_d = nc.declare_dram_parameter("w", [2, 128, 2 * 128], _BF16, isOutput=False)
    bv_d = nc.declare_dram_parameter("bv", [2, 128, NT * HQ], _F32, isOutput=False)
    # y[mh, p, hl, i, w]: output channel = mh*128 + p, local out row = hl*NT + i
    y_d = nc.declare_dram_parameter("y", [2, 128, HQ, NT, WD], _BF16, isOutput=True)

    with tile.TileContext(nc) as tc:
        with (
            tc.tile_pool(name="const", bufs=1) as cpool,
            tc.tile_pool(name="xin", bufs=xin_bufs) as inpool,
            tc.tile_pool(name="obuf", bufs=ob_bufs) as outpool,
            tc.tile_pool(name="ps", bufs=ps_bufs, space="PSUM") as pspool,
        ):
            # consts all on the gpsimd ring, which is otherwise idle at the
            # head (ACT's head is occupied by the auto-inserted activation
            # table load; SP must start input tiles immediately); W per-kh,
            # kh0 first since it gates the first matmul
            # (the stationary matmul operand must be contiguous per partition,
            # so each [128,128] quadrant gets its own tile)
            wt = [
                [cpool.tile([128, 128], _BF16, name=f"w{kh}{mh}") for mh in range(2)]
                for kh in range(2)
            ]
            for mh in range(2):
                for kh in range(2):
                    nc.gpsimd.dma_start(
                        out=wt[kh][mh][:], in_=w_d[kh, :, mh * 128 : (mh + 1) * 128]
                    )
            # bias table on ACT (behind its table load, ready before the
            # first evac); keeping it off Pool keeps the W sems early, and
            # a late bv sem would stall the first evac -> PSUM recycling -> PE
            bvt = [cpool.tile([128, NT * HQ], _F32, name=f"bv{mh}") for mh in range(2)]
            for mh in range(2):
                nc.scalar.dma_start(out=bvt[mh][:], in_=bv_d[mh])

            # PE p-state warmup: pe_busy_start is sticky, so a burst of tiny
            # matmuls during the input-DMA fill window starts the 3us clock
            # ramp early and the real matmuls all run at full rate
            if n_warm:
                wscr = cpool.tile([128, 128], _BF16, name="wscr")
                zscr = cpool.tile([128, 64], _BF16, name="zscr")
                nc.vector.memset(wscr[:], 0.0)
                nc.vector.memset(zscr[:], 0.0)
                warm = pspool.tile([128, SR, WD], _F32, name="warm", tag="ps")
                for _ in range(n_warm):
                    nc.tensor.matmul(warm[:, 0, 0:64], wscr[:], zscr[:], start=True, stop=True)

            slb = {"sp": nc.sync, "act": nc.scalar, "pool": nc.gpsimd}[slb_ring]
            for s in range(NS):
                for i in range(NT):
                    if first_split and s == 0 and i == 0:
                        # first iteration: each kh half in its OWN tile so the
                        # first matmul waits only on its 790ns half-load (tile
                        # dependencies are tile-granular, and each DMA's
                        # completion sem costs +900ns)
                        xk = [
                            inpool.tile([128, SR * WD], _BF16, name=f"xk{kh}", tag=f"xk{kh}")
                            for kh in range(2)
                        ]
                        for kh in range(2):
                            nc.sync.dma_start(out=xk[kh][:], in_=x_d[:, s, i, kh])
                        xsl = lambda kh, a, b: xk[kh][:, a:b]
                    else:
                        xin = inpool.tile([128, 2, SR * WD], _BF16, name=f"x{s}_{i}", tag="xin")
                        nc.sync.dma_start(out=xin[:], in_=x_d[:, s, i])
                        xsl = lambda kh, a, b: xin[:, kh, a:b]
                    last_iter = s == NS - 1 and i == NT - 1
                    for mh in ((1, 0) if last_iter else (0, 1)):
                        ps = pspool.tile([128, SR, WD], _F32, name=f"ps{s}_{i}_{mh}", tag="ps")
                        # bias b[o]*count as per-partition scalar; count is
                        # uniform within an op, so edge stripes split the
                        # boundary row off (count ramps on the outer 11 rows).
                        # s=NS-1 also splits the matmuls/DMA so the kernel's
                        # drain tail ends on a 1-row sliver
                        if s == 0:
                            mm_parts = [(0, SR)]
                            parts = [(0, 1, 0), (1, SR, 1)]
                        elif s == NS - 1:
                            mm_parts = [(0, SR - 1), (SR - 1, SR)]
                            parts = [(0, SR - 1, s * SR), (SR - 1, SR, HQ - 1)]
                        else:
                            mm_parts = [(0, SR)]
                            parts = [(0, SR, s * SR)]
                        # the ISA caps a matmul's moving/out free size at 512
                        # elements (one PSUM bank), so emit 2-row chunks
                        for r0, r1 in mm_parts:
                            for c0 in range(r0, r1, 2):
                                c1 = min(c0 + 2, r1)
                                for j, kh in enumerate((0, 1)):
                                    nc.tensor.matmul(
                                        ps[:, c0:c1], wt[kh][mh][:],
                                        xsl(kh, c0 * WD, c1 * WD),
                                        start=(j == 0), stop=(j == 1),
                                    )
                        ob = outpool.tile([128, SR, WD], _BF16, name=f"ob{s}_{i}_{mh}", tag=f"ob{mh}")
                        for r0, r1, hl in parts:
                            col = i * HQ + hl
                            if mh == 0:
                                nc.scalar.activation(
                                    ob[:, r0:r1],
                                    ps[:, r0:r1],
                                    mybir.ActivationFunctionType.Identity,
                                    bias=bvt[mh][:, col : col + 1],
                                )
                            else:
                                nc.vector.tensor_scalar_add(
                                    ob[:, r0:r1],
                                    ps[:, r0:r1],
                                    bvt[mh][:, col : col + 1],
                                )
                        if last_split and last_iter:
                            # tail: pipeline the final pieces across rings
                            # (the SP ring is drained by now and HWDGE has a
                            # shorter completion-sem lag than SWDGE)
                            ea = nc.scalar if mh == 0 else nc.gpsimd
                            eb = nc.sync if mh == 0 else slb
                            ea.dma_start(
                                out=y_d[mh, :, s * SR : s * SR + SR - 1, i, :],
                                in_=ob[:, 0 : SR - 1],
                            )
                            eb.dma_start(
                                out=y_d[mh, :, s * SR + SR - 1 : (s + 1) * SR, i, :],
                                in_=ob[:, SR - 1 : SR],
                            )
                        else:
                            # the gpsimd/SWDGE completion sem lags ~1.1us
                            # behind the transfer, so the tail-most regular
                            # outputs go on the drained SP ring instead
                            out_eng = (
                                nc.sync
                                if (last_split and s == NS - 1 and i >= NT - sp_tail_n)
                                else nc.gpsimd
                            )
                            out_eng.dma_start(
                                out=y_d[mh, :, s * SR : (s + 1) * SR, i, :],
                                in_=ob[:],
                            )
    nc.finalize()
    _NC_CACHE[key] = nc
    return nc


def _counts() -> np.ndarray:
    """count[r] for output row r (conv-transpose bias multiplicity)."""
    r = np.arange(HOUT)
    return (np.minimum(11, r) - np.maximum(0, r - (HOUT - NT)) + 1).astype(np.float32)


def shard_inputs(inputs: dict) -> list[dict]:
    xs = [np.asarray(inputs[f"x{i}"], dtype=np.float32) for i in range(NT)]
    w = np.asarray(inputs["W"], dtype=np.float32)
    b = np.asarray(inputs["b"], dtype=np.float32)
    counts = _counts()
    # w[kh, k, mh, m] = W[kh*128+k, mh*128+m]
    wp = np.ascontiguousarray(
        w.reshape(2, 128, 2 * 128).astype(_NPBF16)
    )
    in_maps = []
    for cid in range(NCORES):
        b_idx, hq = divmod(cid, 4)
        h0 = hq * HQ
        # x[p, s, i, kh, r*WD] = x_i[b, kh*128+p, h0+s*SR+r, w]
        xp = np.empty((128, NS, NT, 2, SR * WD), dtype=_NPBF16)
        for i in range(NT):
            blk = xs[i][b_idx, :, h0 : h0 + HQ, :]  # [256, 16, 256]
            blk = blk.reshape(2, 128, NS, SR * WD)  # [kh, p, s, rw]
            xp[:, :, i] = blk.transpose(1, 2, 0, 3).astype(_NPBF16)
        # bv[mh, m, i*HQ + hl] = b[mh*128+m] * count(12*(h0+hl) + i)
        i_idx = np.arange(NT)[:, None]
        hl_idx = np.arange(HQ)[None, :]
        cnt = counts[NT * (h0 + hl_idx) + i_idx].reshape(NT * HQ)  # [192]
        bv = (b.reshape(2, 128)[:, :, None] * cnt[None, None, :]).astype(np.float32)
        in_maps.append({"x": xp, "w": wp, "bv": bv})
    return in_maps


def gather_outputs(results: list[dict]) -> np.ndarray:
    out = np.empty((B, C, HOUT, WD), dtype=np.float32)
    for cid in range(NCORES):
        b_idx, hq = divmod(cid, 4)
        h0 = hq * HQ
        # y[mh, p, hl, i, w] -> rows hl*NT+i: exactly the interleave order
        y = np.asarray(results[cid]["y"]).reshape(C, HQ * NT, WD)
        out[b_idx, :, NT * h0 : NT * (h0 + HQ), :] = y.astype(np.float32)
    return out


def kernel(**inputs) -> np.ndarray:
    nc = build_nc()
    in_maps = shard_inputs(inputs)
    res = run_bass_kernel_spmd(nc, in_maps, core_ids=list(range(NCORES)))
    return gather_outputs(res.results)
